# revision 1
# baseline (speedup 1.0000x reference)
"""MoE layer (8 experts, top-2) on 8 Trainium2 NeuronCores.

Expert-parallel: the host runs the tiny router (fp32), gathers each
token to its top-2 experts' batches, and core e runs expert e's FFN
  y = relu(x @ W1[e] + b1[e]) @ W2[e] + b2[e]
over its assigned tokens (padded to a common capacity C). The host then
scales each row by its routing probability and scatter-adds back.

Device kernel (per core, SPMD):
  - weights resident in SBUF as bf16 (W1 64KB/part + W2 64KB/part)
  - tokens processed in chunks of CT=512 columns, transposed layout
    (partition dim = feature dim), PSUM-accumulated matmuls with
    fused bias+ReLU eviction on the scalar engine.
"""

import numpy as np
import ml_dtypes

try:
    import concourse.bass as bass
except ImportError:  # fallback if NIX_PYTHONPATH isn't set up
    import sys

    sys.path.insert(0, "/opt/trn_rl_repo")
    import concourse.bass as bass

import concourse.mybir as mybir
import concourse.tile as tile
from concourse import bacc
from concourse.bass import ts
from concourse.bass_utils import run_bass_kernel_spmd

P = 128
D, H, E, TOPK = 1024, 4096, 8, 2
KD = D // P  # 8 k-tiles over D
MH = H // P  # 32 k-tiles over H
CT = 512  # token-chunk (= matmul free dim = one PSUM bank of f32)

BF16 = ml_dtypes.bfloat16

_NC_CACHE: dict = {}


def _build(C: int, repeat: int = 1):
    """Per-core FFN kernel: yT[:, :, c] = FFN(xT[:, :, c]) in transposed
    (feature-partitioned) layout. repeat>1 re-runs the body for timing."""
    nc = bacc.Bacc()
    AF = mybir.ActivationFunctionType
    xT = nc.dram_tensor("xT", [P, KD, C], mybir.dt.bfloat16, kind="ExternalInput")
    w1 = nc.dram_tensor("w1", [P, KD, H], mybir.dt.bfloat16, kind="ExternalInput")
    w2 = nc.dram_tensor("w2", [P, MH, D], mybir.dt.bfloat16, kind="ExternalInput")
    b1 = nc.dram_tensor("b1", [P, MH], mybir.dt.float32, kind="ExternalInput")
    b2 = nc.dram_tensor("b2", [P, KD], mybir.dt.float32, kind="ExternalInput")
    yT = nc.dram_tensor("yT", [P, KD, C], mybir.dt.float32, kind="ExternalOutput")
    nch = C // CT

    with tile.TileContext(nc) as tc:
        with (
            tc.tile_pool(name="wpool", bufs=1) as wpool,
            tc.tile_pool(name="bpool", bufs=1) as bpool,
            tc.tile_pool(name="xpool", bufs=2) as xpool,
            tc.tile_pool(name="hpool", bufs=1) as hpool,
            tc.tile_pool(name="ypool", bufs=3) as ypool,
            tc.tile_pool(name="psum", bufs=4, space="PSUM") as psum,
        ):
            w1_sb = wpool.tile([P, KD, H], mybir.dt.bfloat16)
            w2_sb = wpool.tile([P, MH, D], mybir.dt.bfloat16)
            b1_sb = bpool.tile([P, MH], mybir.dt.float32)
            b2_sb = bpool.tile([P, KD], mybir.dt.float32)
            nc.sync.dma_start(out=b1_sb[:], in_=b1[:])
            nc.sync.dma_start(out=b2_sb[:], in_=b2[:])
            nc.sync.dma_start(out=w1_sb[:], in_=w1[:])
            nc.sync.dma_start(out=w2_sb[:], in_=w2[:])
            for _ in range(repeat):
                for ch in range(nch):
                    x_sb = xpool.tile([P, KD, CT], mybir.dt.bfloat16)
                    nc.sync.dma_start(out=x_sb[:], in_=xT[:, :, ts(ch, CT)])
                    h_sb = hpool.tile([P, MH, CT], mybir.dt.bfloat16)
                    for m in range(MH):
                        ps = psum.tile([P, CT], mybir.dt.float32)
                        for k in range(KD):
                            nc.tensor.matmul(
                                ps[:],
                                w1_sb[:, k, ts(m, P)],
                                x_sb[:, k, :],
                                start=(k == 0),
                                stop=(k == KD - 1),
                            )
                        nc.scalar.activation(
                            h_sb[:, m, :], ps[:], AF.Relu, bias=b1_sb[:, m : m + 1]
                        )
                    for m in range(KD):
                        ps = psum.tile([P, CT], mybir.dt.float32)
                        for k in range(MH):
                            nc.tensor.matmul(
                                ps[:],
                                w2_sb[:, k, ts(m, P)],
                                h_sb[:, k, :],
                                start=(k == 0),
                                stop=(k == MH - 1),
                            )
                        y_sb = ypool.tile([P, CT], mybir.dt.float32)
                        nc.scalar.activation(
                            y_sb[:], ps[:], AF.Identity, bias=b2_sb[:, m : m + 1]
                        )
                        nc.sync.dma_start(out=yT[:, m, ts(ch, CT)], in_=y_sb[:])
    nc.compile()
    return nc


def _route(x, Wr, br):
    """Host router: per-token top-2 experts + softmax probs (fp32)."""
    xf = np.ascontiguousarray(np.asarray(x, np.float32).reshape(-1, D))
    T = xf.shape[0]
    logits = xf @ np.asarray(Wr, np.float32) + np.asarray(br, np.float32)
    top_i = np.argsort(-logits, axis=-1, kind="stable")[:, :TOPK]  # ties: low idx
    top_v = np.take_along_axis(logits, top_i, axis=-1)
    ex = np.exp(top_v - top_v.max(-1, keepdims=True))
    top_p = ex / ex.sum(-1, keepdims=True)

    ei = top_i.reshape(-1)
    perm = np.argsort(ei, kind="stable")  # assignments sorted by expert
    counts = np.bincount(ei, minlength=E)
    tok = perm // TOPK  # owning token of each sorted assignment
    p_sorted = top_p.reshape(-1)[perm]
    return xf, T, counts, perm, tok, p_sorted


def _prepare_in_maps(xf, counts, tok, W1, b1, W2, b2, C):
    W1 = np.asarray(W1, np.float32)
    W2 = np.asarray(W2, np.float32)
    b1 = np.asarray(b1, np.float32)
    b2 = np.asarray(b2, np.float32)
    offs = np.concatenate([[0], np.cumsum(counts)])
    in_maps = []
    for e in range(E):
        n = counts[e]
        xe = np.zeros((C, D), np.float32)
        xe[:n] = xf[tok[offs[e] : offs[e + 1]]]
        # [p, kd, c] = x[c, kd*P + p]
        xT = np.ascontiguousarray(xe.reshape(C, KD, P).transpose(2, 1, 0)).astype(BF16)
        # [p, kd, h] = W1[kd*P + p, h]
        w1T = np.ascontiguousarray(
            W1[e].reshape(KD, P, H).transpose(1, 0, 2).astype(BF16)
        )
        # [p, kh, d] = W2[kh*P + p, d]
        w2T = np.ascontiguousarray(
            W2[e].reshape(MH, P, D).transpose(1, 0, 2).astype(BF16)
        )
        b1r = np.ascontiguousarray(b1[e].reshape(MH, P).T)
        b2r = np.ascontiguousarray(b2[e].reshape(KD, P).T)
        in_maps.append({"xT": xT, "w1": w1T, "w2": w2T, "b1": b1r, "b2": b2r})
    return in_maps


def _combine(results, counts, perm, p_sorted, T, C, out_shape):
    offs = np.concatenate([[0], np.cumsum(counts)])
    y_sorted = np.empty((int(counts.sum()), D), np.float32)
    for e in range(E):
        n = counts[e]
        if n == 0:
            continue
        yT = results[e]["yT"]  # [P, KD, C] f32
        y_sorted[offs[e] : offs[e + 1]] = (
            yT.transpose(2, 1, 0).reshape(C, D)[:n]
        )
    contrib = np.empty((T * TOPK, D), np.float32)
    contrib[perm] = y_sorted * p_sorted[:, None]
    return contrib.reshape(T, TOPK, D).sum(1).reshape(out_shape)


def _run(x, Wr, br, W1, b1, W2, b2, repeat: int = 1, timings: dict | None = None):
    import time

    xf, T, counts, perm, tok, p_sorted = _route(x, Wr, br)
    C = max(CT, int(-(-counts.max() // CT)) * CT)
    in_maps = _prepare_in_maps(xf, counts, tok, W1, b1, W2, b2, C)

    key = (C, repeat)
    if key not in _NC_CACHE:
        _NC_CACHE[key] = _build(C, repeat)
    nc = _NC_CACHE[key]

    t0 = time.time()
    res = run_bass_kernel_spmd(nc, in_maps, core_ids=list(range(E)))
    t1 = time.time()
    if timings is not None:
        timings["run_wall"] = t1 - t0
    out = _combine(res.results, counts, perm, p_sorted, T, C, np.asarray(x).shape)
    return out


def kernel(x, Wr, br, W1, b1, W2, b2):
    return _run(x, Wr, br, W1, b1, W2, b2).astype(np.float32)


# revision 4
# speedup vs baseline: 560.9673x; 560.9673x over previous
"""MoE layer (8 experts, top-2) on 8 Trainium2 NeuronCores.

Expert-parallel: the host runs the tiny router (fp32), gathers each
token to its top-2 experts' batches, and core e runs expert e's FFN
  y_pre = relu(x @ W1[e] + b1[e]) @ W2[e]
over its assigned tokens (padded to a common capacity C). The host
adds b2, scales each row by its routing probability, and scatter-adds
back into the output.

Device kernel (per core, SPMD):
  - W1/W2 resident in SBUF as bf16 (64KB/partition each)
  - tokens processed in chunks of <=512 columns (one PSUM bank);
    layer 1 keeps weights stationary and streams token columns
    (output hT is feature-partitioned), layer 2 keeps 128-token tiles
    of hT stationary and streams W2 columns (output y is
    token-partitioned, so the host needs no transpose)
  - bias+ReLU fused into the PSUM->SBUF eviction on the scalar engine
"""

import numpy as np
import ml_dtypes

try:
    import concourse.bass as bass
except ImportError:  # fallback if NIX_PYTHONPATH isn't set up
    import sys

    sys.path.insert(0, "/opt/trn_rl_repo")
    import concourse.bass as bass

import concourse.mybir as mybir
import concourse.tile as tile
from concourse import bacc
from concourse.bass import ts, ds
from concourse.bass_utils import run_bass_kernel_spmd

P = 128
D, H, E, TOPK = 1024, 4096, 8, 2
KD = D // P  # 8 k-tiles over D
MH = H // P  # 32 k-tiles over H
CT = 512  # max token-chunk (= matmul free dim = one PSUM bank of f32)

BF16 = ml_dtypes.bfloat16

_NC_CACHE: dict = {}


def _chunks(C):
    out = []
    off = 0
    while off < C:
        cs = min(CT, C - off)
        out.append((off, cs))
        off += cs
    return out


def _build(C: int, repeat: int = 1):
    assert C % P == 0
    nc = bacc.Bacc()
    AF = mybir.ActivationFunctionType
    xT = nc.dram_tensor("xT", [P, KD, C], mybir.dt.bfloat16, kind="ExternalInput")
    w1 = nc.dram_tensor("w1", [P, KD, H], mybir.dt.bfloat16, kind="ExternalInput")
    w2 = nc.dram_tensor("w2", [P, MH, D], mybir.dt.bfloat16, kind="ExternalInput")
    b1 = nc.dram_tensor("b1", [P, MH], mybir.dt.float32, kind="ExternalInput")
    y = nc.dram_tensor("y", [C // P, P, D], mybir.dt.float32, kind="ExternalOutput")

    with tile.TileContext(nc) as tc:
        with (
            tc.tile_pool(name="wpool", bufs=1) as wpool,
            tc.tile_pool(name="bpool", bufs=1) as bpool,
            tc.tile_pool(name="xpool", bufs=2) as xpool,
            tc.tile_pool(name="hpool", bufs=1) as hpool,
            tc.tile_pool(name="ypool", bufs=2) as ypool,
            tc.tile_pool(name="ps1", bufs=2, space="PSUM") as ps1_pool,
            tc.tile_pool(name="ps2", bufs=2, space="PSUM") as ps2_pool,
        ):
            w1_sb = wpool.tile([P, KD, H], mybir.dt.bfloat16)
            w2_sb = wpool.tile([P, MH, D], mybir.dt.bfloat16)
            b1_sb = bpool.tile([P, MH], mybir.dt.float32)
            # first chunk's tokens land before the weight stream starts
            x_first = xpool.tile([P, KD, CT], mybir.dt.bfloat16)
            cs0 = min(CT, C)
            nc.sync.dma_start(
                out=x_first[:, 0 : KD // 2, :cs0], in_=xT[:, 0 : KD // 2, ds(0, cs0)]
            )
            nc.sync.dma_start(
                out=x_first[:, KD // 2 :, :cs0], in_=xT[:, KD // 2 :, ds(0, cs0)]
            )
            nc.sync.dma_start(out=b1_sb[:], in_=b1[:])
            # weights stream in slices across parallel DMA queues so the
            # first matmuls start as soon as slice 0 arrives
            for s in range(8):
                nc.sync.dma_start(
                    out=w1_sb[:, :, ts(s, H // 8)], in_=w1[:, :, ts(s, H // 8)]
                )
            for s in range(4):
                nc.sync.dma_start(
                    out=w2_sb[:, ts(s, MH // 4), :], in_=w2[:, ts(s, MH // 4), :]
                )
            first = True
            for _ in range(repeat):
                for off, cs in _chunks(C):
                    if first:
                        x_sb, first = x_first, False
                    else:
                        x_sb = xpool.tile([P, KD, CT], mybir.dt.bfloat16)
                        nc.sync.dma_start(
                            out=x_sb[:, :, :cs], in_=xT[:, :, ds(off, cs)]
                        )
                    h_sb = hpool.tile([P, MH, CT], mybir.dt.bfloat16)
                    # layer 1: hT[m, c] = relu(sum_k W1[k,m]^T x[k,c] + b1[m])
                    for m in range(MH):
                        ps = ps1_pool.tile([P, CT], mybir.dt.float32)
                        for k in range(KD):
                            nc.tensor.matmul(
                                ps[:, :cs],
                                w1_sb[:, k, ts(m, P)],
                                x_sb[:, k, :cs],
                                start=(k == 0),
                                stop=(k == KD - 1),
                            )
                        nc.scalar.activation(
                            h_sb[:, m, :cs], ps[:, :cs], AF.Relu,
                            bias=b1_sb[:, m : m + 1],
                        )
                    # layer 2: y[t, d] = sum_k hT[k, t]^T W2[k, d]
                    # (hT 128-token tiles stationary, W2 columns moving)
                    for t in range(cs // P):
                        ps = ps2_pool.tile([P, D], mybir.dt.float32)
                        for k in range(MH):
                            ht = h_sb[:, k, ts(t, P)]
                            for nb in range(D // CT):
                                nc.tensor.matmul(
                                    ps[:, ts(nb, CT)],
                                    ht,
                                    w2_sb[:, k, ts(nb, CT)],
                                    start=(k == 0),
                                    stop=(k == MH - 1),
                                )
                        y_sb = ypool.tile([P, D], mybir.dt.float32)
                        nc.scalar.activation(y_sb[:], ps[:], AF.Copy)
                        nc.sync.dma_start(out=y[off // P + t], in_=y_sb[:])
    nc.compile()
    return nc


def _route(x, Wr, br):
    """Host router: per-token top-2 experts + softmax probs (fp32)."""
    xf = np.ascontiguousarray(np.asarray(x, np.float32).reshape(-1, D))
    T = xf.shape[0]
    logits = xf @ np.asarray(Wr, np.float32) + np.asarray(br, np.float32)
    top_i = np.argsort(-logits, axis=-1, kind="stable")[:, :TOPK]  # ties: low idx
    top_v = np.take_along_axis(logits, top_i, axis=-1)
    ex = np.exp(top_v - top_v.max(-1, keepdims=True))
    top_p = ex / ex.sum(-1, keepdims=True)

    ei = top_i.reshape(-1)
    perm = np.argsort(ei, kind="stable")  # assignments sorted by expert
    counts = np.bincount(ei, minlength=E)
    tok = perm // TOPK  # owning token of each sorted assignment
    p_sorted = top_p.reshape(-1)[perm]
    return xf, T, counts, perm, tok, p_sorted


def _prepare_in_maps(xf, counts, tok, W1, b1, W2, C):
    W1 = np.asarray(W1, np.float32)
    W2 = np.asarray(W2, np.float32)
    b1 = np.asarray(b1, np.float32)
    offs = np.concatenate([[0], np.cumsum(counts)])
    in_maps = []
    for e in range(E):
        n = counts[e]
        xe = np.zeros((C, D), np.float32)
        xe[:n] = xf[tok[offs[e] : offs[e + 1]]]
        # [p, kd, c] = x[c, kd*P + p]
        xT = np.ascontiguousarray(xe.reshape(C, KD, P).transpose(2, 1, 0)).astype(BF16)
        # [p, kd, h] = W1[kd*P + p, h]
        w1T = np.ascontiguousarray(
            W1[e].reshape(KD, P, H).transpose(1, 0, 2).astype(BF16)
        )
        # [p, kh, d] = W2[kh*P + p, d]
        w2T = np.ascontiguousarray(
            W2[e].reshape(MH, P, D).transpose(1, 0, 2).astype(BF16)
        )
        b1r = np.ascontiguousarray(b1[e].reshape(MH, P).T)
        in_maps.append({"xT": xT, "w1": w1T, "w2": w2T, "b1": b1r})
    return in_maps


def _combine(results, counts, perm, p_sorted, b2, T, C, out_shape):
    b2 = np.asarray(b2, np.float32)
    offs = np.concatenate([[0], np.cumsum(counts)])
    y_sorted = np.empty((int(counts.sum()), D), np.float32)
    for e in range(E):
        n = counts[e]
        if n == 0:
            continue
        ye = results[e]["y"].reshape(C, D)[:n]  # [C//P, P, D] -> [C, D]
        y_sorted[offs[e] : offs[e + 1]] = ye + b2[e]
    contrib = np.empty((T * TOPK, D), np.float32)
    contrib[perm] = y_sorted * p_sorted[:, None]
    return contrib.reshape(T, TOPK, D).sum(1).reshape(out_shape)


def _run(x, Wr, br, W1, b1, W2, b2, repeat: int = 1, timings: dict | None = None):
    import time

    xf, T, counts, perm, tok, p_sorted = _route(x, Wr, br)
    C = max(P, int(-(-int(counts.max()) // P)) * P)
    in_maps = _prepare_in_maps(xf, counts, tok, W1, b1, W2, C)

    key = (C, repeat)
    if key not in _NC_CACHE:
        _NC_CACHE[key] = _build(C, repeat)
    nc = _NC_CACHE[key]

    t0 = time.time()
    res = run_bass_kernel_spmd(nc, in_maps, core_ids=list(range(E)))
    t1 = time.time()
    if timings is not None:
        timings["run_wall"] = t1 - t0
    out = _combine(
        res.results, counts, perm, p_sorted, b2, T, C, np.asarray(x).shape
    )
    return out


def kernel(x, Wr, br, W1, b1, W2, b2):
    return _run(x, Wr, br, W1, b1, W2, b2).astype(np.float32)


# revision 5
# speedup vs baseline: 607.7422x; 1.0834x over previous
"""MoE layer (8 experts, top-2) on 8 Trainium2 NeuronCores — paired-expert load-balanced expert parallelism.

Experts are paired (largest with smallest by token count); each pair is
hosted by two cores, each core computing half of both experts' tokens.
Capacity per core = SA + SB where SA/SB are the uniform padded segment
sizes for the pair's larger/smaller expert — less padding waste than
one-expert-per-core when loads are imbalanced.

Device kernel: W2 for BOTH slots resident in SBUF (128KB/partition);
W1 streamed per 128-column m-block (2KB/partition each, prefetched);
otherwise identical to kernel.py's chunked two-layer structure.
"""

import numpy as np
import ml_dtypes

try:
    import concourse.bass as bass
except ImportError:
    import sys

    sys.path.insert(0, "/opt/trn_rl_repo")
    import concourse.bass as bass

import concourse.mybir as mybir
import concourse.tile as tile
from concourse import bacc
from concourse.bass import ts, ds
from concourse.bass_utils import run_bass_kernel_spmd

P = 128
D, H, E, TOPK = 1024, 4096, 8, 2
KD = D // P
MH = H // P
CT = 512

BF16 = ml_dtypes.bfloat16

_NC_CACHE: dict = {}


def _seg_chunks(base, seg, slot):
    out = []
    off = 0
    while off < seg:
        cs = min(CT, seg - off)
        out.append((base + off, cs, slot))
        off += cs
    return out


def _build(SA: int, SB: int, repeat: int = 1):
    C = SA + SB
    assert C % P == 0 and SA % P == 0 and SB % P == 0
    nc = bacc.Bacc()
    AF = mybir.ActivationFunctionType
    xT = nc.dram_tensor("xT", [P, KD, C], mybir.dt.bfloat16, kind="ExternalInput")
    # w1 pre-tiled per m-block for streaming: [slot, m, p, k, col]
    w1 = nc.dram_tensor("w1", [2, MH, P, KD, P], mybir.dt.bfloat16, kind="ExternalInput")
    w2 = nc.dram_tensor("w2", [P, 2, MH, D], mybir.dt.bfloat16, kind="ExternalInput")
    b1 = nc.dram_tensor("b1", [P, 2, MH], mybir.dt.float32, kind="ExternalInput")
    y = nc.dram_tensor("y", [C // P, P, D], mybir.dt.float32, kind="ExternalOutput")

    chunks = _seg_chunks(0, SA, 0) + _seg_chunks(SA, SB, 1)

    with tile.TileContext(nc) as tc:
        with (
            tc.tile_pool(name="w2pool", bufs=1) as w2pool,
            tc.tile_pool(name="w1pool", bufs=4) as w1pool,
            tc.tile_pool(name="bpool", bufs=1) as bpool,
            tc.tile_pool(name="xpool", bufs=2) as xpool,
            tc.tile_pool(name="hpool", bufs=1) as hpool,
            tc.tile_pool(name="ypool", bufs=1) as ypool,
            tc.tile_pool(name="ps1", bufs=2, space="PSUM") as ps1_pool,
            tc.tile_pool(name="ps2", bufs=2, space="PSUM") as ps2_pool,
        ):
            w2_sb = w2pool.tile([P, 2, MH, D], mybir.dt.bfloat16)
            b1_sb = bpool.tile([P, 2, MH], mybir.dt.float32)
            # first chunk's tokens land first, then W2 streams in slices
            x_first = xpool.tile([P, KD, CT], mybir.dt.bfloat16)
            cs0 = chunks[0][1]
            nc.sync.dma_start(
                out=x_first[:, 0 : KD // 2, :cs0], in_=xT[:, 0 : KD // 2, ds(0, cs0)]
            )
            nc.sync.dma_start(
                out=x_first[:, KD // 2 :, :cs0], in_=xT[:, KD // 2 :, ds(0, cs0)]
            )
            nc.sync.dma_start(out=b1_sb[:], in_=b1[:])

            def _load_w2():
                # emitted after the first chunk's layer-1 so the streamed
                # W1 blocks win the DMA queues at kernel start
                for sl in range(2):
                    for s in range(4):
                        nc.sync.dma_start(
                            out=w2_sb[:, sl, ts(s, MH // 4), :],
                            in_=w2[:, sl, ts(s, MH // 4), :],
                        )

            first = True
            w2_loaded = False
            for _ in range(repeat):
                for off, cs, slot in chunks:
                    if first:
                        x_sb, first = x_first, False
                    else:
                        x_sb = xpool.tile([P, KD, CT], mybir.dt.bfloat16)
                        nc.sync.dma_start(
                            out=x_sb[:, :, :cs], in_=xT[:, :, ds(off, cs)]
                        )
                    h_sb = hpool.tile([P, MH, CT], mybir.dt.bfloat16)
                    for m in range(MH):
                        w1_blk = w1pool.tile([P, KD, P], mybir.dt.bfloat16)
                        nc.sync.dma_start(out=w1_blk[:], in_=w1[slot, m])
                        ps = ps1_pool.tile([P, CT], mybir.dt.float32)
                        for k in range(KD):
                            nc.tensor.matmul(
                                ps[:, :cs],
                                w1_blk[:, k, :],
                                x_sb[:, k, :cs],
                                start=(k == 0),
                                stop=(k == KD - 1),
                            )
                        nc.scalar.activation(
                            h_sb[:, m, :cs], ps[:, :cs], AF.Relu,
                            bias=b1_sb[:, slot, m : m + 1],
                        )
                    if not w2_loaded:
                        _load_w2()
                        w2_loaded = True
                    for t in range(cs // P):
                        ps = ps2_pool.tile([P, D], mybir.dt.float32)
                        for k in range(MH):
                            ht = h_sb[:, k, ts(t, P)]
                            for nb in range(D // CT):
                                nc.tensor.matmul(
                                    ps[:, ts(nb, CT)],
                                    ht,
                                    w2_sb[:, slot, k, ts(nb, CT)],
                                    start=(k == 0),
                                    stop=(k == MH - 1),
                                )
                        y_sb = ypool.tile([P, D], mybir.dt.float32)
                        nc.scalar.activation(y_sb[:], ps[:], AF.Copy)
                        nc.sync.dma_start(out=y[off // P + t], in_=y_sb[:])
    nc.compile()
    return nc


def _route(x, Wr, br):
    xf = np.ascontiguousarray(np.asarray(x, np.float32).reshape(-1, D))
    T = xf.shape[0]
    logits = xf @ np.asarray(Wr, np.float32) + np.asarray(br, np.float32)
    top_i = np.argsort(-logits, axis=-1, kind="stable")[:, :TOPK]
    top_v = np.take_along_axis(logits, top_i, axis=-1)
    ex = np.exp(top_v - top_v.max(-1, keepdims=True))
    top_p = ex / ex.sum(-1, keepdims=True)

    ei = top_i.reshape(-1)
    perm = np.argsort(ei, kind="stable")
    counts = np.bincount(ei, minlength=E)
    tok = perm // TOPK
    p_sorted = top_p.reshape(-1)[perm]
    return xf, T, counts, perm, tok, p_sorted


def _plan(counts):
    """Pair experts (largest with smallest); split each expert's tokens in
    half across the pair's two cores. Returns uniform SA/SB and per-core
    (expertA, pieceA_start, pieceA_len, expertB, pieceB_start, pieceB_len)."""
    order = np.argsort(-counts, kind="stable")
    pairs = [(int(order[i]), int(order[E - 1 - i])) for i in range(E // 2)]

    def rup(n):
        return -(-int(n) // P) * P

    SA = max(P, max(rup(-(-counts[a] // 2)) for a, _ in pairs))
    SB = max(P, max(rup(-(-counts[b] // 2)) for _, b in pairs))
    cores = []
    for a, b in pairs:
        a1 = int(-(-counts[a] // 2))
        b1 = int(-(-counts[b] // 2))
        cores.append((a, 0, a1, b, 0, b1))
        cores.append((a, a1, int(counts[a]) - a1, b, b1, int(counts[b]) - b1))
    return SA, SB, cores


def _prepare_in_maps(xf, counts, tok, W1, b1, W2, SA, SB, cores):
    W1 = np.asarray(W1, np.float32)
    W2 = np.asarray(W2, np.float32)
    b1 = np.asarray(b1, np.float32)
    offs = np.concatenate([[0], np.cumsum(counts)])
    C = SA + SB
    # per-expert prepped weights (shared between the pair's two cores)
    w1t = {}
    w2t = {}
    b1t = {}
    for e in set(int(c[0]) for c in cores) | set(int(c[3]) for c in cores):
        # [m, p, k, col] = W1[k*P+p, m*P+col]
        w1t[e] = np.ascontiguousarray(
            W1[e].reshape(KD, P, MH, P).transpose(2, 1, 0, 3).astype(BF16)
        )
        w2t[e] = np.ascontiguousarray(
            W2[e].reshape(MH, P, D).transpose(1, 0, 2).astype(BF16)
        )
        b1t[e] = np.ascontiguousarray(b1[e].reshape(MH, P).T)
    in_maps = []
    for a, sa, la, b, sb, lb in cores:
        xe = np.zeros((C, D), np.float32)
        xe[:la] = xf[tok[offs[a] + sa : offs[a] + sa + la]]
        xe[SA : SA + lb] = xf[tok[offs[b] + sb : offs[b] + sb + lb]]
        xT = np.ascontiguousarray(xe.reshape(C, KD, P).transpose(2, 1, 0)).astype(BF16)
        in_maps.append(
            {
                "xT": xT,
                "w1": np.stack([w1t[a], w1t[b]]),
                "w2": np.ascontiguousarray(
                    np.stack([w2t[a], w2t[b]], axis=1)
                ),
                "b1": np.ascontiguousarray(np.stack([b1t[a], b1t[b]], axis=1)),
            }
        )
    return in_maps


def _combine(results, counts, perm, p_sorted, b2, T, SA, SB, cores, out_shape):
    b2 = np.asarray(b2, np.float32)
    offs = np.concatenate([[0], np.cumsum(counts)])
    C = SA + SB
    y_sorted = np.empty((int(counts.sum()), D), np.float32)
    for core, (a, sa, la, b, sb, lb) in enumerate(cores):
        ye = results[core]["y"].reshape(C, D)
        if la:
            y_sorted[offs[a] + sa : offs[a] + sa + la] = ye[:la] + b2[a]
        if lb:
            y_sorted[offs[b] + sb : offs[b] + sb + lb] = ye[SA : SA + lb] + b2[b]
    contrib = np.empty((T * TOPK, D), np.float32)
    contrib[perm] = y_sorted * p_sorted[:, None]
    return contrib.reshape(T, TOPK, D).sum(1).reshape(out_shape)


def _run(x, Wr, br, W1, b1, W2, b2, repeat: int = 1, timings: dict | None = None):
    import time

    xf, T, counts, perm, tok, p_sorted = _route(x, Wr, br)
    SA, SB, cores = _plan(counts)
    in_maps = _prepare_in_maps(xf, counts, tok, W1, b1, W2, SA, SB, cores)

    key = (SA, SB, repeat)
    if key not in _NC_CACHE:
        _NC_CACHE[key] = _build(SA, SB, repeat)
    nc = _NC_CACHE[key]

    t0 = time.time()
    res = run_bass_kernel_spmd(nc, in_maps, core_ids=list(range(E)))
    t1 = time.time()
    if timings is not None:
        timings["run_wall"] = t1 - t0
    return _combine(
        res.results, counts, perm, p_sorted, b2, T, SA, SB, cores,
        np.asarray(x).shape,
    )


def kernel(x, Wr, br, W1, b1, W2, b2):
    return _run(x, Wr, br, W1, b1, W2, b2).astype(np.float32)
